# revision 14
# baseline (speedup 1.0000x reference)
"""Trainium2 Bass kernel for nn_Block_78022375899354 (dense transformer block).

Sharding (8 cores): core c -> batch b=c//2, head-half hh=c%2.
  Phase 1 (self-attn): head-split — each core computes q/k/v for its 8 heads over
    the full batch, causal attention, and a partial attention-projection which is
    pairwise ReduceScattered over token halves.
  Phase 2+3 (cross-attn, MLP, adapter): token-split — each core owns 512 tokens.
All activations feature-major [channels on partitions, tokens on free dim].
Matmuls in bf16 with f32 PSUM accumulation; residual stream in f32.
LayerNorm gain/bias and all projection biases are folded host-side
(mathematically exact: ln(x)*g+b @ W = lnraw(x) @ (g*W) + (b@W); v-bias flows
through softmax as an exact additive term since probs sum to 1).
"""
import sys
sys.path.insert(0, '/opt/trn_rl_repo')
import numpy as np
import ml_dtypes

BF = ml_dtypes.bfloat16
P = 128
C = 1024
T = 1024
TE = 257
TEP = 384          # padded encoder length (3 chunks of 128)
NCH = C // P       # 8 channel chunks
F = 512            # free-dim tile (tokens)
H = 16
D = 64
EPS = 1e-5

_BUILT = {}


def _build_nc():
    import concourse.bass as bass
    import concourse.mybir as mybir
    import concourse.tile as tile
    from contextlib import ExitStack

    f32 = mybir.dt.float32
    bf16 = mybir.dt.bfloat16
    AF = mybir.ActivationFunctionType
    ALU = mybir.AluOpType

    nc = bass.Bass("TRN2", num_devices=8)

    # ---------------- DRAM I/O ----------------
    xT = nc.dram_tensor("xT", [C, T], f32, kind="ExternalInput")
    x_ownT = nc.dram_tensor("x_ownT", [C, F], f32, kind="ExternalInput")
    encT = nc.dram_tensor("encT", [C, TEP], bf16, kind="ExternalInput")
    wqkv = nc.dram_tensor("wqkv", [C, 1536], bf16, kind="ExternalInput")
    bqk = nc.dram_tensor("bqk", [1024], f32, kind="ExternalInput")
    waproj = nc.dram_tensor("waproj", [512, C], bf16, kind="ExternalInput")
    battn = nc.dram_tensor("battn", [C], f32, kind="ExternalInput")
    wca = nc.dram_tensor("wca", [C, 3 * C], bf16, kind="ExternalInput")
    bcaqk = nc.dram_tensor("bcaqk", [2 * C], f32, kind="ExternalInput")
    wcaproj = nc.dram_tensor("wcaproj", [C, C], bf16, kind="ExternalInput")
    bcaproj = nc.dram_tensor("bcaproj", [C], f32, kind="ExternalInput")
    wfc = nc.dram_tensor("wfc", [C, 4 * C], bf16, kind="ExternalInput")
    bfc = nc.dram_tensor("bfc", [4 * C], f32, kind="ExternalInput")
    wmproj = nc.dram_tensor("wmproj", [NCH, P, 4 * C], bf16, kind="ExternalInput")
    bmproj = nc.dram_tensor("bmproj", [C], f32, kind="ExternalInput")
    wdown = nc.dram_tensor("wdown", [C, 256], bf16, kind="ExternalInput")
    bdown = nc.dram_tensor("bdown", [256], f32, kind="ExternalInput")
    wup = nc.dram_tensor("wup", [256, C], bf16, kind="ExternalInput")
    bup = nc.dram_tensor("bup", [C], f32, kind="ExternalInput")
    out_d = nc.dram_tensor("out", [C, F], f32, kind="ExternalOutput")

    def r3(ap):
        return ap.rearrange("(o p) f -> p o f", p=P)

    def r2(ap):
        return ap.rearrange("(o p) -> p o", p=P)

    with tile.TileContext(nc) as tc:
        with ExitStack() as ctx:
            consts = ctx.enter_context(tc.tile_pool(name="consts", bufs=1))
            work = ctx.enter_context(tc.tile_pool(name="work", bufs=2))
            lns = ctx.enter_context(tc.tile_pool(name="lns", bufs=1))
            wstream = ctx.enter_context(tc.tile_pool(name="wstream", bufs=3))
            dram = ctx.enter_context(tc.tile_pool(name="dram", bufs=1, space="DRAM"))
            ps_main = ctx.enter_context(
                tc.tile_pool(name="ps_main", bufs=5, space="PSUM"))
            ps_aux = ctx.enter_context(
                tc.tile_pool(name="ps_aux", bufs=3, space="PSUM"))
            x2pool = ctx.enter_context(tc.tile_pool(name="x2pool", bufs=1))
            lnxb_pool = ctx.enter_context(tc.tile_pool(name="lnxb_pool", bufs=2))
            exp_pool = ctx.enter_context(tc.tile_pool(name="exp_pool", bufs=9))

            # ---------- constants ----------
            ones_col_bf = consts.tile([P, 1], bf16)
            nc.vector.memset(ones_col_bf, 1.0)
            ones_row_bf = consts.tile([1, P], bf16)
            nc.vector.memset(ones_row_bf, 1.0)
            ones_row_f32 = consts.tile([1, P], f32)
            nc.vector.memset(ones_row_f32, 1.0)
            # causal diagonal-offset masks: masks[i, k, j] = 1 iff j >= i + 128*k
            masks = consts.tile([P, 4, F], bf16)
            for k in range(4):
                nc.gpsimd.memset(masks[:, k, :], 1.0)
                nc.gpsimd.affine_select(
                    out=masks[:, k, :], in_=masks[:, k, :],
                    compare_op=ALU.is_ge, fill=0.0,
                    base=-(P * k), channel_multiplier=-1, pattern=[[1, F]],
                )
            # encoder pad handling: rows>0 of kt-chunk 2 get exp bias -1e30 -> 0
            padbias = consts.tile([P, 1], f32)
            nc.vector.memset(padbias, -1e30)
            nc.vector.memset(padbias[0:1, :], 0.0)
            eps_sb = consts.tile([1, 1], f32)
            nc.vector.memset(eps_sb, EPS)

            # ---------- bias tiles ----------
            def bias_tile(dr, ncols):
                t = consts.tile([P, ncols], f32)
                nc.sync.dma_start(t, r2(dr[:]))
                return t
            bqk_sb = bias_tile(bqk, 8)
            battn_sb = bias_tile(battn, NCH)
            bcaqk_sb = bias_tile(bcaqk, 16)
            bcaproj_sb = bias_tile(bcaproj, NCH)
            bfc_sb = bias_tile(bfc, 32)
            bmproj_sb = bias_tile(bmproj, NCH)
            bdown_sb = bias_tile(bdown, 2)
            bup_sb = bias_tile(bup, NCH)

            # ---------- layernorm (feature-major, pipelined 2-pass) ----------
            def layernorm(x_sb, ntok, ln_out, cast_fn=None):
                stats = []
                for nt in range(ntok // F):
                    sl = slice(nt * F, (nt + 1) * F)
                    s1 = ps_main.tile([1, F], f32, tag="acc")
                    s2 = ps_main.tile([1, F], f32, tag="acc")
                    xb_all = lnxb_pool.tile([P, NCH, F], bf16)
                    for kc in range(NCH):
                        if cast_fn is not None:
                            cast_fn(xb_all[:, kc], kc, sl)
                        # split casts across ACT and GpSimd to halve latency
                        elif kc % 2 == 0:
                            nc.scalar.copy(xb_all[:, kc], x_sb[:, kc, sl])
                        else:
                            nc.gpsimd.tensor_copy(xb_all[:, kc], x_sb[:, kc, sl])
                    for kc in range(NCH):
                        nc.tensor.matmul(s1, ones_col_bf, xb_all[:, kc],
                                         start=(kc == 0), stop=(kc == NCH - 1))
                    for kc in range(NCH):
                        xsq = work.tile([P, F], bf16, tag="lnxsq")
                        nc.vector.tensor_mul(xsq, xb_all[:, kc], xb_all[:, kc])
                        nc.tensor.matmul(s2, ones_col_bf, xsq,
                                         start=(kc == 0), stop=(kc == NCH - 1))
                    stats.append((sl, s1, s2, xb_all))
                for sl, s1, s2, xb_all in stats:
                    mrow = lns.tile([1, F], f32, tag="m")
                    nc.vector.tensor_scalar_mul(mrow, s1, 1.0 / C)
                    var = lns.tile([1, F], f32, tag="v")
                    nc.vector.tensor_scalar_mul(var, s2, 1.0 / C)
                    msq = lns.tile([1, F], f32, tag="msq")
                    nc.vector.tensor_mul(msq, mrow, mrow)
                    nc.vector.tensor_sub(var, var, msq)
                    srow = lns.tile([1, F], f32, tag="s")
                    nc.scalar.activation(srow, var, AF.Sqrt, bias=eps_sb[:, 0:1])
                    rstd = lns.tile([1, F], bf16, tag="r")
                    with nc.allow_low_precision(reason="ln rstd bf16"):
                        nc.vector.reciprocal(rstd, srow)
                    brow = lns.tile([1, F], bf16, tag="b")
                    nc.vector.scalar_tensor_tensor(
                        brow, in0=mrow, scalar=-1.0, in1=rstd,
                        op0=ALU.mult, op1=ALU.mult)
                    psA = ps_main.tile([P, F], f32, tag="acc")
                    psB = ps_main.tile([P, F], f32, tag="acc")
                    nc.tensor.matmul(psA, ones_row_bf, rstd, start=True, stop=True)
                    nc.tensor.matmul(psB, ones_row_bf, brow, start=True, stop=True)
                    A_sb = work.tile([P, F], bf16, tag="lnA")
                    nc.scalar.copy(A_sb, psA)
                    B_sb = work.tile([P, F], bf16, tag="lnB")
                    nc.scalar.copy(B_sb, psB)
                    # all-bf16 SBUF ops hit the DVE fast mode
                    for kc in range(NCH):
                        tmp = work.tile([P, F], bf16, tag="lntmp")
                        nc.vector.tensor_mul(tmp, xb_all[:, kc], A_sb)
                        nc.vector.tensor_add(ln_out[:, kc, sl], tmp, B_sb)

            # attention softmax-normalize: attn_out[0:64] = pav[0:64] * (1/pav[64])
            def attn_norm(pav, dst):
                r = lns.tile([1, F], bf16, tag="recip")
                with nc.allow_low_precision(reason="softmax recip bf16"):
                    nc.vector.reciprocal(r, pav[64:65, :])
                pbc = ps_aux.tile([64, F], f32, tag="aux")
                nc.tensor.matmul(pbc, ones_row_bf[:, :64], r, start=True, stop=True)
                rb = work.tile([64, F], bf16, tag="rbc")
                nc.vector.tensor_copy(rb, pbc)
                nc.vector.tensor_mul(dst, pav[0:64, :], rb)

            cc_in = dram.tile([2, C, F], bf16)
            cc_out = dram.tile([C, F], bf16)

            # =================================================================
            # Phase 1: self-attention (head-split, full batch)
            # =================================================================
            with ExitStack() as p1:
                pool_ln1 = p1.enter_context(tc.tile_pool(name="pool_ln1", bufs=1))
                ln1T = pool_ln1.tile([P, NCH, T], bf16)
                with tc.tile_pool(name="pool_x", bufs=1) as pool_x:
                    xT_sb = pool_x.tile([P, NCH, T], f32)
                    xr = r3(xT[:])
                    for kc in range(NCH):
                        nc.sync.dma_start(xT_sb[:, kc], xr[:, kc])
                    layernorm(xT_sb, T, ln1T)

                pool_p1 = p1.enter_context(tc.tile_pool(name="pool_p1", bufs=1))
                wqkv_sb = pool_p1.tile([P, NCH, 1536], bf16)
                nc.sync.dma_start(wqkv_sb, r3(wqkv[:]))

                q_sb = pool_p1.tile([P, 4, T], bf16)
                k_sb = pool_p1.tile([P, 4, T], bf16)
                for m in range(4):
                    for ntk in range(T // F):
                        for dst, woff, boff in ((q_sb, 0, 0), (k_sb, 512, 4)):
                            pt = ps_main.tile([P, F], f32, tag="acc")
                            for kc in range(NCH):
                                nc.tensor.matmul(
                                    pt,
                                    wqkv_sb[:, kc, woff + m * P:woff + (m + 1) * P],
                                    ln1T[:, kc, ntk * F:(ntk + 1) * F],
                                    start=(kc == 0), stop=(kc == NCH - 1))
                            nc.scalar.activation(
                                dst[:, m, ntk * F:(ntk + 1) * F], pt, AF.Identity,
                                bias=bqk_sb[:, boff + m:boff + m + 1])

                v_sb = pool_p1.tile([P, NCH, 8, 65], bf16)
                nc.vector.memset(v_sb[:, :, :, 64:65], 1.0)
                for tkc in range(NCH):
                    pt = ps_main.tile([P, F], f32, tag="acc")
                    for kc in range(NCH):
                        nc.tensor.matmul(pt, ln1T[:, kc, tkc * P:(tkc + 1) * P],
                                         wqkv_sb[:, kc, 1024:1536],
                                         start=(kc == 0), stop=(kc == NCH - 1))
                    nc.vector.tensor_copy(
                        v_sb[:, tkc, :, 0:64],
                        pt.rearrange("p (h d) -> p h d", h=8))

                attn_sb = pool_p1.tile([P, 4, T], bf16)
                pending = None
                for hl in range(8):
                    pb = (hl % 2) * 64
                    hch = hl // 2
                    for qt in range(2):
                        nkc = 4 * (qt + 1)
                        pav = ps_aux.tile([65, F], f32, tag="aux")
                        es = []
                        for kc in range(nkc):
                            ps_s = ps_main.tile([P, F], f32, tag="acc")
                            nc.tensor.matmul(
                                ps_s,
                                k_sb[pb:pb + 64, hch, kc * P:(kc + 1) * P],
                                q_sb[pb:pb + 64, hch, qt * F:(qt + 1) * F],
                                start=True, stop=True)
                            e = exp_pool.tile([P, F], bf16, tag="exp")
                            nc.scalar.activation(e, ps_s, AF.Exp, scale=0.125)
                            dk = kc - 4 * qt
                            if dk >= 0:
                                nc.vector.tensor_mul(e, e, masks[:, dk, :])
                            es.append((kc, e))
                        for kc, e in es:
                            nc.tensor.matmul(pav, v_sb[:, kc, hl, :], e,
                                             start=(kc == 0),
                                             stop=(kc == nkc - 1))
                        if pending is not None:
                            attn_norm(*pending)
                        pending = (pav,
                                   attn_sb[pb:pb + 64, hch, qt * F:(qt + 1) * F])
                attn_norm(*pending)

                waproj_sb = pool_p1.tile([P, 4, C], bf16)
                nc.sync.dma_start(waproj_sb, r3(waproj[:]))
                for qt in range(2):
                    for m in range(NCH):
                        pt = ps_main.tile([P, F], f32, tag="acc")
                        for kc in range(4):
                            nc.tensor.matmul(
                                pt, waproj_sb[:, kc, m * P:(m + 1) * P],
                                attn_sb[:, kc, qt * F:(qt + 1) * F],
                                start=(kc == 0), stop=(kc == 3))
                        part = work.tile([P, F], bf16, tag="part")
                        nc.vector.tensor_copy(part, pt)
                        nc.sync.dma_start(cc_in[qt, m * P:(m + 1) * P, :], part)

            x2 = x2pool.tile([P, NCH, F], f32)

            # =================================================================
            # Phase 2: cross-attention (token-split, own 512 tokens)
            # =================================================================
            with ExitStack() as p2:
                pool_p2 = p2.enter_context(tc.tile_pool(name="pool_p2", bufs=1))
                # encoder K/V is independent of the collective result:
                # compute it here so PE stays busy during the ReduceScatter.
                encT_sb = pool_p2.tile([P, NCH, TEP], bf16)
                nc.sync.dma_start(encT_sb, r3(encT[:]))
                kc_sb = pool_p2.tile([P, NCH, TEP], bf16)
                wca_k = wstream.tile([P, NCH, C], bf16, tag="w8k")
                nc.sync.dma_start(wca_k, r3(wca[:, C:2 * C]))
                for m in range(NCH):
                    pt = ps_main.tile([P, TEP], f32, tag="acc")
                    for kc in range(NCH):
                        nc.tensor.matmul(pt, wca_k[:, kc, m * P:(m + 1) * P],
                                         encT_sb[:, kc, :],
                                         start=(kc == 0), stop=(kc == NCH - 1))
                    nc.scalar.activation(kc_sb[:, m, :], pt, AF.Identity,
                                         bias=bcaqk_sb[:, 8 + m:8 + m + 1])
                vc_sb = pool_p2.tile([P, 3, H, 65], bf16)
                nc.vector.memset(vc_sb[:, :, :, 64:65], 1.0)
                wca_v = wstream.tile([P, NCH, C], bf16, tag="w8k")
                nc.sync.dma_start(wca_v, r3(wca[:, 2 * C:3 * C]))
                for tkc in range(3):
                    for nh in range(2):
                        pt = ps_main.tile([P, F], f32, tag="acc")
                        for kc in range(NCH):
                            nc.tensor.matmul(
                                pt, encT_sb[:, kc, tkc * P:(tkc + 1) * P],
                                wca_v[:, kc, nh * F:(nh + 1) * F],
                                start=(kc == 0), stop=(kc == NCH - 1))
                        nc.vector.tensor_copy(
                            vc_sb[:, tkc, nh * 8:(nh + 1) * 8, 0:64],
                            pt.rearrange("p (h d) -> p h d", h=8))

                nc.gpsimd.collective_compute(
                    "ReduceScatter", ALU.add,
                    replica_groups=[[0, 1], [2, 3], [4, 5], [6, 7]],
                    ins=[cc_in[:]], outs=[cc_out[:]])

                # x_own = x + attn_out (RS) + combined attn bias
                x_own = pool_p2.tile([P, NCH, F], f32)
                rs_sb = pool_p2.tile([P, NCH, F], bf16)
                ccr = r3(cc_out[:])
                for kc in range(NCH):
                    nc.sync.dma_start(rs_sb[:, kc], ccr[:, kc])
                xin_sb = pool_p2.tile([P, NCH, F], f32)
                nc.sync.dma_start(xin_sb, r3(x_ownT[:]))

                def cast_x_own(dst, kc, sl):
                    # bf16 stats input computed straight from RS result
                    nc.vector.scalar_tensor_tensor(
                        dst, in0=rs_sb[:, kc, :], scalar=battn_sb[:, kc:kc + 1],
                        in1=xin_sb[:, kc, :], op0=ALU.add, op1=ALU.add)

                ln2T = pool_p2.tile([P, NCH, F], bf16)
                layernorm(x_own, F, ln2T, cast_fn=cast_x_own)
                # f32 residual (consumed ~40us later at the caproj drain)
                for m in range(NCH):
                    nc.vector.scalar_tensor_tensor(
                        x_own[:, m, :], in0=rs_sb[:, m, :],
                        scalar=battn_sb[:, m:m + 1], in1=xin_sb[:, m, :],
                        op0=ALU.add, op1=ALU.add)

                qc_sb = pool_p2.tile([P, NCH, F], bf16)
                wca_q = wstream.tile([P, NCH, C], bf16, tag="w8k")
                nc.sync.dma_start(wca_q, r3(wca[:, 0:C]))
                for m in range(NCH):
                    pt = ps_main.tile([P, F], f32, tag="acc")
                    for kc in range(NCH):
                        nc.tensor.matmul(pt, wca_q[:, kc, m * P:(m + 1) * P],
                                         ln2T[:, kc, :],
                                         start=(kc == 0), stop=(kc == NCH - 1))
                    nc.scalar.activation(qc_sb[:, m, :], pt, AF.Identity,
                                         bias=bcaqk_sb[:, m:m + 1])

                attnc_sb = pool_p2.tile([P, NCH, F], bf16)
                pending = None
                for h in range(H):
                    pb = (h % 2) * 64
                    hch = h // 2
                    pav = ps_aux.tile([65, F], f32, tag="aux")
                    es = []
                    for kc in range(3):
                        ps_s = ps_main.tile([P, F], f32, tag="acc")
                        nc.tensor.matmul(
                            ps_s, kc_sb[pb:pb + 64, hch, kc * P:(kc + 1) * P],
                            qc_sb[pb:pb + 64, hch, :], start=True, stop=True)
                        e = exp_pool.tile([P, F], bf16, tag="exp")
                        if kc == 2:
                            nc.scalar.activation(e, ps_s, AF.Exp, scale=0.125,
                                                 bias=padbias[:, 0:1])
                        else:
                            nc.scalar.activation(e, ps_s, AF.Exp, scale=0.125)
                        es.append((kc, e))
                    for kc, e in es:
                        nc.tensor.matmul(pav, vc_sb[:, kc, h, :], e,
                                         start=(kc == 0), stop=(kc == 2))
                    if pending is not None:
                        attn_norm(*pending)
                    pending = (pav, attnc_sb[pb:pb + 64, hch, :])
                attn_norm(*pending)

                wcaproj_sb = wstream.tile([P, NCH, C], bf16, tag="w8k")
                nc.sync.dma_start(wcaproj_sb, r3(wcaproj[:]))
                for m in range(NCH):
                    pt = ps_main.tile([P, F], f32, tag="acc")
                    for kc in range(NCH):
                        nc.tensor.matmul(pt, wcaproj_sb[:, kc, m * P:(m + 1) * P],
                                         attnc_sb[:, kc, :],
                                         start=(kc == 0), stop=(kc == NCH - 1))
                    nc.vector.scalar_tensor_tensor(
                        x2[:, m, :], in0=pt, scalar=bcaproj_sb[:, m:m + 1],
                        in1=x_own[:, m, :], op0=ALU.add, op1=ALU.add)

            # =================================================================
            # Phase 3: MLP + adapter (token-split)
            # =================================================================
            with ExitStack() as p3:
                pool_p3 = p3.enter_context(tc.tile_pool(name="pool_p3", bufs=1))
                ln3T = pool_p3.tile([P, NCH, F], bf16)
                layernorm(x2, F, ln3T)

                gT = pool_p3.tile([P, 32, F], bf16)
                for quarter in range(4):
                    wfc_t = wstream.tile([P, NCH, C], bf16, tag="w8k")
                    nc.sync.dma_start(wfc_t, r3(wfc[:, quarter * C:(quarter + 1) * C]))
                    for m8 in range(8):
                        m = quarter * 8 + m8
                        pt = ps_main.tile([P, F], f32, tag="acc")
                        for kc in range(NCH):
                            nc.tensor.matmul(pt, wfc_t[:, kc, m8 * P:(m8 + 1) * P],
                                             ln3T[:, kc, :],
                                             start=(kc == 0), stop=(kc == NCH - 1))
                        nc.scalar.activation(gT[:, m, :], pt, AF.Gelu_apprx_tanh,
                                             bias=bfc_sb[:, m:m + 1])

                h_sb = pool_p3.tile([P, NCH, F], bf16)
                wmp_pool = p3.enter_context(tc.tile_pool(name="wmp_pool", bufs=3))
                for m in range(NCH):
                    # stream the column block of wmproj for output chunk m
                    wmp_t = wmp_pool.tile([P, 32, P], bf16, tag="wmp")
                    nc.sync.dma_start(
                        wmp_t, wmproj[m].rearrange("p (o f) -> p o f", f=P))
                    pt = ps_main.tile([P, F], f32, tag="acc")
                    for kc in range(32):
                        nc.tensor.matmul(pt, wmp_t[:, kc, :], gT[:, kc, :],
                                         start=(kc == 0), stop=(kc == 31))
                    nc.scalar.activation(h_sb[:, m, :], pt, AF.Identity,
                                         bias=bmproj_sb[:, m:m + 1])

                wdown_sb = pool_p3.tile([P, NCH, 256], bf16)
                nc.sync.dma_start(wdown_sb, r3(wdown[:]))
                wup_sb = pool_p3.tile([P, 2, C], bf16)
                nc.sync.dma_start(wup_sb, r3(wup[:]))

                aT = pool_p3.tile([P, 2, F], bf16)
                for m in range(2):
                    pt = ps_main.tile([P, F], f32, tag="acc")
                    for kc in range(NCH):
                        nc.tensor.matmul(pt, wdown_sb[:, kc, m * P:(m + 1) * P],
                                         h_sb[:, kc, :],
                                         start=(kc == 0), stop=(kc == NCH - 1))
                    nc.scalar.activation(aT[:, m, :], pt, AF.Gelu_apprx_tanh,
                                         bias=bdown_sb[:, m:m + 1])

                for m in range(NCH):
                    pt = ps_main.tile([P, F], f32, tag="acc")
                    for kc in range(2):
                        nc.tensor.matmul(pt, wup_sb[:, kc, m * P:(m + 1) * P],
                                         aT[:, kc, :], start=(kc == 0), stop=(kc == 1))
                    tmp = work.tile([P, F], f32, tag="fin")
                    nc.vector.scalar_tensor_tensor(
                        tmp, in0=pt, scalar=bup_sb[:, m:m + 1], in1=h_sb[:, m, :],
                        op0=ALU.add, op1=ALU.add)
                    fin = work.tile([P, F], f32, tag="fin2")
                    nc.vector.tensor_add(fin, tmp, x2[:, m, :])
                    nc.sync.dma_start(out_d[m * P:(m + 1) * P, :], fin)

    _split_sync_waits(nc, mybir)
    return nc


def _split_sync_waits(nc, mybir, maxw=1):
    # walrus rejects instructions with more than a couple of sync waits
    # (e.g. the Tile epilogue Drain waits on every engine + DMA queue);
    # move excess waits onto preceding same-engine no-ops.
    for f in nc.m.functions:
        for bb in f.blocks:
            out, changed = [], False
            for ins in bb.instructions:
                si = ins.sync_info
                if si is not None and len(si.on_wait) > maxw:
                    waits = list(si.on_wait)
                    k = 0
                    while len(waits) > maxw:
                        chunk, waits = waits[:maxw], waits[maxw:]
                        nop = mybir.InstNoOp(name=f"{ins.name}-w{k}", ins=[], outs=[])
                        nop.engine = ins.engine
                        nop.sync_info = mybir.SyncInfo(on_wait=chunk, on_update=[])
                        out.append(nop)
                        k += 1
                    ins.sync_info = mybir.SyncInfo(
                        on_wait=waits, on_update=list(si.on_update))
                    changed = True
                out.append(ins)
            if changed:
                bb.instructions = out


def _prep_inputs(inputs):
    f = lambda k: np.asarray(inputs[k], np.float32)
    x = f('x')
    enc = f('encoder_embd')
    ln1_g, ln1_b = f('ln1_g'), f('ln1_b')
    ln2_g, ln2_b = f('ln2_g'), f('ln2_b')
    ln3_g, ln3_b = f('ln3_g'), f('ln3_b')
    attn_w, attn_b = f('attn_w'), f('attn_b')
    aproj_w, aproj_b = f('aproj_w'), f('aproj_b')
    ca_w, ca_b = f('ca_w'), f('ca_b')
    caproj_w, caproj_b = f('caproj_w'), f('caproj_b')
    fc_w, fc_b = f('fc_w'), f('fc_b')
    mproj_w, mproj_b = f('mproj_w'), f('mproj_b')
    down_w, down_b = f('down_w'), f('down_b')
    up_w, up_b = f('up_w'), f('up_b')

    # fold LN affine into consuming weights (exact for g=1,b=0 fills)
    aw = ln1_g[:, None] * attn_w
    ab = ln1_b @ attn_w + attn_b
    caw_q = ln2_g[:, None] * ca_w[:, :C]
    cab_q = ln2_b @ ca_w[:, :C] + ca_b[:C]
    fw = ln3_g[:, None] * fc_w
    fb = ln3_b @ fc_w + fc_b

    battn = aproj_b + ab[2 * C:] @ aproj_w            # v-bias folded (probs sum to 1)
    bcaproj = caproj_b + ca_b[2 * C:] @ caproj_w

    wca_full = np.concatenate([caw_q, ca_w[:, C:2 * C], ca_w[:, 2 * C:]], 1).astype(BF)
    bcaqk = np.concatenate([cab_q, ca_b[C:2 * C]]).astype(np.float32)

    shared = dict(
        wca=wca_full, bcaqk=bcaqk,
        wcaproj=caproj_w.astype(BF), bcaproj=bcaproj.astype(np.float32),
        wfc=fw.astype(BF), bfc=fb.astype(np.float32),
        wmproj=np.ascontiguousarray(
            mproj_w.reshape(32, P, NCH, P).transpose(2, 1, 0, 3)
        ).reshape(NCH, P, 4 * C).astype(BF),
        bmproj=mproj_b.astype(np.float32),
        wdown=down_w.astype(BF), bdown=down_b.astype(np.float32),
        wup=up_w.astype(BF), bup=up_b.astype(np.float32),
        battn=battn.astype(np.float32),
    )

    in_maps = []
    for c in range(8):
        b, hh = c // 2, c % 2
        hs = slice(hh * 512, hh * 512 + 512)
        wqkv = np.concatenate([aw[:, hs], aw[:, C:2 * C][:, hs],
                               aw[:, 2 * C:][:, hs]], 1)
        bqk = np.concatenate([ab[hs], ab[C:2 * C][hs]])
        encp = np.zeros((TEP, C), np.float32)
        encp[:TE] = enc[b]
        xTb = np.ascontiguousarray(x[b].T)
        m = dict(shared)
        m.update(
            xT=xTb,
            x_ownT=np.ascontiguousarray(xTb[:, hh * F:(hh + 1) * F]),
            encT=np.ascontiguousarray(encp.T).astype(BF),
            wqkv=wqkv.astype(BF),
            bqk=bqk.astype(np.float32),
            waproj=aproj_w[hs].astype(BF),
        )
        in_maps.append(m)
    return in_maps


def kernel(**inputs):
    from concourse.bass_utils import run_bass_kernel_spmd
    if 'nc' not in _BUILT:
        _BUILT['nc'] = _build_nc()
    in_maps = _prep_inputs(inputs)
    res = run_bass_kernel_spmd(_BUILT['nc'], in_maps, core_ids=list(range(8)))
    y = np.zeros((4, T, C), np.float32)
    for c in range(8):
        b, half = c // 2, c % 2
        y[b, half * F:(half + 1) * F, :] = res.results[c]["out"].T
    return y
